# revision 9
# baseline (speedup 1.0000x reference)
"""MoE sparse routing (TT-compressed experts) Trainium2 kernel.

Strategy
--------
The reference applies, per sample b, a rank-32 TT-matrix (6 TT cores,
expert-mixed by hard gumbel-softmax gates) to X[b] in R^{512x768}.
Because the gates are (numerically) one-hot, the whole TT chain
collapses per expert e into two small matrices:

    P_e [32, 768]  (input side:  cores 0..2)
    Q_e [768, 32]  (output side: cores 3..5)
    y_b = (X_b @ P_e^T) @ Q_e^T * 16     for e = argmax(logits_b + gumbel)

On device we build P/Q for ALL 4 experts (stacked along a 128-wide
rank-expert axis), compute T4^T = P4T^T-chain for every expert at once
(full 128-wide PE usage), scale T4^T rows by the broadcast one-hot gate
(fused into the PSUM->SBUF copy), and contract against the stacked
Q4T [128, 768] so the expert mixing happens inside the second matmul.

Sharding: pure data-parallel over batch: 32 samples -> 8 NeuronCores,
4 samples each.  TT cores / router weights are replicated.  No
collectives.
"""

import numpy as np

B, S, MDIM = 32, 512, 768
E = 4
N_CORES = 8
B_LOC = B // N_CORES  # 4
ALPHA = 16.0
INV_S = 1.0 / 512.0  # exact power of two
KC = 6  # 768 / 128 k-chunks
SC = 4  # 512 / 128 s-chunks


def _build_kernel():
    from contextlib import ExitStack

    import concourse.bass as bass
    import concourse.tile as tile
    import concourse.mybir as mybir
    from concourse import bacc
    from concourse.masks import make_identity

    fp32 = mybir.dt.float32
    AF = mybir.ActivationFunctionType
    ALU = mybir.AluOpType

    nc = bacc.Bacc(
        "TRN2", target_bir_lowering=False, debug=False, enable_asserts=False
    )

    X = nc.dram_tensor("X", [B_LOC, S, MDIM], fp32, kind="ExternalInput").ap()
    U = nc.dram_tensor("U", [B_LOC, E], fp32, kind="ExternalInput").ap()
    RW = nc.dram_tensor("RW", [E, MDIM], fp32, kind="ExternalInput").ap()
    RB = nc.dram_tensor("RB", [E], fp32, kind="ExternalInput").ap()
    C0 = nc.dram_tensor("C0", [E, 1, 12, 32], fp32, kind="ExternalInput").ap()
    C1 = nc.dram_tensor("C1", [E, 32, 8, 32], fp32, kind="ExternalInput").ap()
    C2 = nc.dram_tensor("C2", [E, 32, 8, 32], fp32, kind="ExternalInput").ap()
    C3 = nc.dram_tensor("C3", [E, 32, 8, 32], fp32, kind="ExternalInput").ap()
    C4 = nc.dram_tensor("C4", [E, 32, 8, 32], fp32, kind="ExternalInput").ap()
    C5 = nc.dram_tensor("C5", [E, 32, 12, 1], fp32, kind="ExternalInput").ap()
    Y = nc.dram_tensor("Y", [B_LOC, S, MDIM], fp32, kind="ExternalOutput").ap()

    with tile.TileContext(nc) as tc, ExitStack() as ctx:
        consts = ctx.enter_context(tc.tile_pool(name="consts", bufs=1))
        xnat_pool = ctx.enter_context(tc.tile_pool(name="xnat", bufs=4))
        xt_pool = ctx.enter_context(tc.tile_pool(name="xt", bufs=24))
        t4g_pool = ctx.enter_context(tc.tile_pool(name="t4g", bufs=2))
        y_pool = ctx.enter_context(tc.tile_pool(name="ysb", bufs=2))

        ps_build = ctx.enter_context(tc.tile_pool(name="psb", bufs=1, space="PSUM"))
        ps_xt = ctx.enter_context(tc.tile_pool(name="psxt", bufs=2, space="PSUM"))
        ps_t4 = ctx.enter_context(tc.tile_pool(name="pst4", bufs=2, space="PSUM"))
        ps_y = ctx.enter_context(tc.tile_pool(name="psy", bufs=2, space="PSUM"))
        ps_cs = ctx.enter_context(tc.tile_pool(name="pscs", bufs=1, space="PSUM"))

        # ---------------- constants & small inputs ----------------
        ident = consts.tile([128, 128], fp32)
        make_identity(nc, ident)
        oneS = consts.tile([128, 1], fp32)
        nc.vector.memset(oneS, INV_S)
        # kronK[e', (e,r)] = 1 iff e == e'; built 32-aligned then PE-transposed
        kronT = consts.tile([128, E], fp32)
        nc.vector.memset(kronT, 0.0)
        for e in range(E):
            nc.vector.memset(kronT[32 * e : 32 * e + 32, e : e + 1], 1.0)
        ps_kron = ps_cs.tile([E, 128], fp32, tag="pscs")
        nc.tensor.transpose(ps_kron, kronT, ident)
        kronK = consts.tile([E, 128], fp32)
        nc.vector.tensor_copy(kronK, ps_kron)

        rw_sb = consts.tile([E, MDIM], fp32)
        nc.sync.dma_start(out=rw_sb, in_=RW)
        u_sb = consts.tile([B_LOC, E], fp32)
        nc.sync.dma_start(out=u_sb, in_=U)
        rb_sb = consts.tile([B_LOC, E], fp32)
        nc.gpsimd.dma_start(
            out=rb_sb,
            in_=bass.AP(tensor=RB.tensor, offset=RB.offset, ap=[[0, B_LOC]] + RB.ap),
        )

        g0 = consts.tile([12, E, 32], fp32)  # [c, e, p1]
        nc.sync.dma_start(out=g0, in_=C0.rearrange("e o c p -> c (e o) p"))
        g1 = consts.tile([32, E, 8, 32], fp32)  # [p1, e, b, p2]
        nc.sync.dma_start(out=g1, in_=C1.rearrange("e r b p -> r e b p"))
        g2 = consts.tile([32, E, 8, 32], fp32)  # [p2, e, a, p3]
        nc.sync.dma_start(out=g2, in_=C2.rearrange("e r a p -> r e a p"))
        g3 = consts.tile([32, E, 8, 32], fp32)  # [r3, e, n0, p4]
        nc.sync.dma_start(out=g3, in_=C3.rearrange("e r n p -> r e n p"))
        g4 = consts.tile([32, E, 8, 32], fp32)  # [p4, e, n1, p5]
        nc.sync.dma_start(out=g4, in_=C4.rearrange("e r n p -> r e n p"))
        g5 = consts.tile([32, E, 12], fp32)  # [p5, e, n2]
        nc.sync.dma_start(out=g5, in_=C5.rearrange("e r n o -> r e (n o)"))

        # ---------------- kick off the X loads early ----------------
        x_nat = []
        for b in range(B_LOC):
            xb = xnat_pool.tile([128, SC, MDIM], fp32, tag="xnat")
            nc.sync.dma_start(out=xb, in_=X[b].rearrange("(c p) k -> p c k", p=128))
            x_nat.append(xb)

        # ---------------- build P4T / Q4T from the TT cores ----------------
        # t0[p1, e, c] = core0[e, 0, c, p1]
        ps = ps_build.tile([32, E, 12], fp32, tag="bps")
        for e in range(E):
            nc.tensor.transpose(ps[:, e, :], g0[:, e, :], ident[:12, :12])
        t0_sb = consts.tile([32, E, 12], fp32)
        nc.vector.tensor_copy(t0_sb, ps)

        # A^T[p2, e, b, c] = sum_p1 G1[p1,b,p2] * G0[c,p1]
        ps = ps_build.tile([32, E, 8, 12], fp32, tag="bps")
        for e in range(E):
            for b in range(8):
                nc.tensor.matmul(
                    ps[:, e, b, :], g1[:, e, b, :], t0_sb[:, e, :],
                    start=True, stop=True,
                )
        a4t = consts.tile([32, E, 8, 12], fp32)
        nc.vector.tensor_copy(a4t, ps)

        # P4[(e,p3), k] = sum_p2 G2[p2,a,p3] * A^T[p2,(b,c)]   (k = a*96+b*12+c)
        p4 = consts.tile([128, MDIM], fp32)
        for half in range(2):
            ps = ps_build.tile([128, 384], fp32, tag="bps")
            for e in range(E):
                for q in range(4):
                    nc.tensor.matmul(
                        ps[32 * e : 32 * e + 32, 96 * q : 96 * q + 96],
                        g2[:, e, 4 * half + q, :],
                        a4t[:, e, :, :],
                        start=True, stop=True,
                        tile_position=(0, 32 * e),
                    )
            nc.vector.tensor_copy(p4[:, 384 * half : 384 * (half + 1)], ps)

        # P4T[k % 128, j, (e,p3)] — transposed chunks for mm1 lhsT
        p4t = consts.tile([128, KC, 128], fp32)
        for j in range(KC):
            ps = ps_build.tile([128, 128], fp32, tag="bps")
            nc.tensor.transpose(ps, p4[:, 128 * j : 128 * (j + 1)], ident)
            nc.vector.tensor_copy(p4t[:, j, :], ps)

        # g4t[p5, e, n1, p4] (transposed 32x32 blocks of core4)
        g4t = consts.tile([32, E, 8, 32], fp32)
        for h in range(2):
            ps = ps_build.tile([32, E, 4, 32], fp32, tag="bps")
            for e in range(E):
                for q in range(4):
                    nc.tensor.transpose(
                        ps[:, e, q, :], g4[:, e, 4 * h + q, :], ident[:32, :32]
                    )
            nc.vector.tensor_copy(g4t[:, :, 4 * h : 4 * h + 4, :], ps)

        # g3t[p4, e, n0, r3]
        g3t = consts.tile([32, E, 8, 32], fp32)
        for h in range(2):
            ps = ps_build.tile([32, E, 4, 32], fp32, tag="bps")
            for e in range(E):
                for q in range(4):
                    nc.tensor.transpose(
                        ps[:, e, q, :], g3[:, e, 4 * h + q, :], ident[:32, :32]
                    )
            nc.vector.tensor_copy(g3t[:, :, 4 * h : 4 * h + 4, :], ps)

        # B5[p4, e, n1, n2] = sum_p5 G4[p4,n1,p5] * G5[p5,n2]
        ps = ps_build.tile([32, E, 8, 12], fp32, tag="bps")
        for e in range(E):
            for n1 in range(8):
                nc.tensor.matmul(
                    ps[:, e, n1, :], g4t[:, e, n1, :], g5[:, e, :],
                    start=True, stop=True,
                )
        b5 = consts.tile([32, E, 8, 12], fp32)
        nc.vector.tensor_copy(b5, ps)

        # Q4T[(e,r3), n] = 16 * sum_p4 G3[r3,n0,p4] * B5[p4,(n1,n2)]
        q4t = consts.tile([128, MDIM], fp32)
        for half in range(2):
            ps = ps_build.tile([128, 384], fp32, tag="bps")
            for e in range(E):
                for q in range(4):
                    nc.tensor.matmul(
                        ps[32 * e : 32 * e + 32, 96 * q : 96 * q + 96],
                        g3t[:, e, 4 * half + q, :],
                        b5[:, e, :, :],
                        start=True, stop=True,
                        tile_position=(0, 32 * e),
                    )
            nc.vector.tensor_scalar(
                q4t[:, 384 * half : 384 * (half + 1)], ps, ALPHA, None, op0=ALU.mult
            )

        # rwt[k % 128, j, e] — router weights transposed
        ps = ps_build.tile([128, KC, E], fp32, tag="bps")
        for j in range(KC):
            nc.tensor.transpose(
                ps[:, j, :], rw_sb[:, 128 * j : 128 * (j + 1)], ident[:4, :4]
            )
        rwt = consts.tile([128, KC, E], fp32)
        nc.vector.tensor_copy(rwt, ps)

        # ---------------- phase 1: transpose X, pooled sums ----------------
        cs_sb = consts.tile([128, KC, B_LOC], fp32)  # pooled[k, j, b]
        x_t = []  # x_t[b][j] : [128, 512] = X[b]^T chunk
        for b in range(B_LOC):
            xtb = []
            for j in range(KC):
                ps = ps_xt.tile([128, 512], fp32, tag="psxt")
                for c in range(SC):
                    nc.tensor.transpose(
                        ps[:, 128 * c : 128 * (c + 1)],
                        x_nat[b][:, c, 128 * j : 128 * (j + 1)],
                        ident,
                    )
                xt = xt_pool.tile([128, 512], fp32, tag="xt")
                if j < 4:
                    nc.vector.tensor_copy(xt, ps)
                else:
                    nc.scalar.copy(xt, ps)
                xtb.append(xt)
            x_t.append(xtb)

            psc = ps_cs.tile([128, KC], fp32, tag="pscs")
            for j in range(KC):
                for c in range(SC):
                    nc.tensor.matmul(
                        psc[:, j : j + 1],
                        x_nat[b][:, c, 128 * j : 128 * (j + 1)],
                        oneS,
                        start=(c == 0), stop=(c == SC - 1),
                    )
            nc.vector.tensor_copy(cs_sb[:, :, b], psc)

        # ---------------- phase 2: router ----------------
        ps_lg = ps_cs.tile([B_LOC, E], fp32, tag="pscs")
        for j in range(KC):
            nc.tensor.matmul(
                ps_lg, cs_sb[:, j, :], rwt[:, j, :],
                start=(j == 0), stop=(j == KC - 1),
            )
        # gumbel noise: g = -ln(-ln(u + 1e-10) + 1e-10); z = logits + rb + g
        eps_b = consts.tile([B_LOC, 1], fp32)
        nc.vector.memset(eps_b, 1e-10)
        t1 = consts.tile([B_LOC, E], fp32)
        nc.scalar.activation(t1, u_sb, AF.Ln, bias=eps_b, scale=1.0)
        t2 = consts.tile([B_LOC, E], fp32)
        nc.scalar.activation(t2, t1, AF.Ln, bias=eps_b, scale=-1.0)
        za = consts.tile([B_LOC, E], fp32)
        nc.vector.tensor_add(za, ps_lg, rb_sb)
        z = consts.tile([B_LOC, E], fp32)
        nc.vector.tensor_sub(z, za, t2)
        zmax = consts.tile([B_LOC, 1], fp32)
        nc.vector.reduce_max(zmax, z, axis=mybir.AxisListType.X)
        gates = consts.tile([B_LOC, E], fp32)
        nc.vector.tensor_scalar(gates, z, zmax, None, op0=ALU.is_ge)
        # g_bcast[(e,r), b] = gates[b, e]
        ps_gt = ps_cs.tile([E, B_LOC], fp32, tag="pscs")
        nc.tensor.transpose(ps_gt, gates, ident[:4, :4])
        gt_sb = consts.tile([E, B_LOC], fp32)
        nc.vector.tensor_copy(gt_sb, ps_gt)
        ps_gb = ps_cs.tile([128, B_LOC], fp32, tag="pscs")
        nc.tensor.matmul(ps_gb, kronK, gt_sb, start=True, stop=True)
        gb_sb = consts.tile([128, B_LOC], fp32)
        nc.vector.tensor_copy(gb_sb, ps_gb)

        # ---------------- phase 3: per-sample expert matmuls ----------------
        for b in range(B_LOC):
            pst4 = ps_t4.tile([128, 512], fp32, tag="t4")
            for j in range(KC):
                nc.tensor.matmul(
                    pst4, p4t[:, j, :], x_t[b][j],
                    start=(j == 0), stop=(j == KC - 1),
                )
            t4g = t4g_pool.tile([128, 512], fp32, tag="t4g")
            nc.vector.tensor_scalar(
                t4g, pst4, gb_sb[:, b : b + 1], None, op0=ALU.mult
            )

            y_sb = y_pool.tile([128, SC, MDIM], fp32, tag="ysb")
            for sc in range(SC):
                for h in range(2):
                    psy = ps_y.tile([128, 384], fp32, tag="psy")
                    nc.tensor.matmul(
                        psy,
                        t4g[:, 128 * sc : 128 * (sc + 1)],
                        q4t[:, 384 * h : 384 * (h + 1)],
                        start=True, stop=True,
                    )
                    if h == 0:
                        nc.vector.tensor_copy(
                            y_sb[:, sc, 384 * h : 384 * (h + 1)], psy
                        )
                    else:
                        nc.scalar.copy(y_sb[:, sc, 384 * h : 384 * (h + 1)], psy)
            nc.sync.dma_start(
                out=Y[b].rearrange("(c p) k -> p c k", p=128), in_=y_sb
            )

    nc.compile()
    return nc


_CACHE = {}

# test-harness knobs (ignored by the grading harness, which calls kernel())
TRACE = False
TRACE_KWARGS = {}
LAST_RESULT = None


def _get_nc():
    if "nc" not in _CACHE:
        _CACHE["nc"] = _build_kernel()
    return _CACHE["nc"]


def kernel(X, router_w, router_b, u_noise, core0, core1, core2, core3, core4, core5):
    from concourse.bass_utils import run_bass_kernel_spmd

    X = np.ascontiguousarray(X, dtype=np.float32)
    u_noise = np.ascontiguousarray(u_noise, dtype=np.float32)
    shared = {
        "RW": np.ascontiguousarray(router_w, dtype=np.float32),
        "RB": np.ascontiguousarray(router_b, dtype=np.float32),
        "C0": np.ascontiguousarray(core0, dtype=np.float32),
        "C1": np.ascontiguousarray(core1, dtype=np.float32),
        "C2": np.ascontiguousarray(core2, dtype=np.float32),
        "C3": np.ascontiguousarray(core3, dtype=np.float32),
        "C4": np.ascontiguousarray(core4, dtype=np.float32),
        "C5": np.ascontiguousarray(core5, dtype=np.float32),
    }
    in_maps = []
    for i in range(N_CORES):
        sl = slice(i * B_LOC, (i + 1) * B_LOC)
        in_maps.append({"X": X[sl], "U": u_noise[sl], **shared})

    nc = _get_nc()
    res = run_bass_kernel_spmd(
        nc, in_maps, list(range(N_CORES)), trace=TRACE, **TRACE_KWARGS
    )
    global LAST_RESULT
    LAST_RESULT = res
    return np.concatenate([r["Y"] for r in res.results], axis=0)
